# revision 1
# baseline (speedup 1.0000x reference)
"""Trainium2 Bass kernel for nn_NetworkODEModel (gnn_message_passing), v2.

Math (per batch b, node i, neighbors j):
  node_out = MLP_n(x)                                      (tiny)
  z1 = u_i + v'_j          u = x@cW1a, v' = x@cW1b + cb1
  c1 = leaky(z1) = 0.99*relu(z1) + 0.01*z1
  z2 = c1@cW2 + cb2
     = 0.99*relu(z1)@cW2 + 0.01*v'@cW2 + (0.01*u@cW2 + cb2)
       [ mm1 blkW99 ]       [ mm2 blkW01 ]    [ U2cb bias ]
  q  = leaky(z2)   (ACT Lrelu, alpha=0.01, bias=U2cb column)
  S_i = sum_j A_ij * q_ij ; coupling = S@cWo + rowsum(A)*cbo

Sharding: i-rows split across 8 cores (32 each), batch whole.  Groups of
2 (b,i) pairs stacked on 128 partitions (HC=64 each), j=256 on free dim.
Per group: DVE ts-ptr relu (4x bf16), 2 block-diagonal 128x128 matmuls,
then either
  ACT path: ACT Lrelu (PSUM->SBUF, 398ns) + DVE copy+accum (4x, 127ns)
  DVE path (g%16 in {0,6,11}): DVE z2+bias+accum s1 (392ns) + DVE
      min+accum s2 (127ns); S = s1 - 0.99*s2 fixed up in the epilogue.
The split balances ACT (~323ns/group) against DVE (~327ns/group).

A-row handling:
 - fast path (rows of A constant off-diagonal, true for A_p=0): S column
   accumulates sum_j q unweighted; the diagonal term q_ii is computed on
   the host (exact, tiny) and subtracted in the epilogue, then the column
   is scaled by the per-row constant a_i.
 - general path: per-group DVE tensor_tensor_reduce against the broadcast
   A row (diag already zeroed); no diagonal correction needed.

Scheduling: inputs are packed into 6 combined DMA loads (each dma_start
costs ~650ns of serialized SP sequencer time), ordered by first use; a
few warm-up matmuls start the PE p-state ramp during the DMA wait; the
main loop runs bp-major so only the first 512-col slice of vT gates the
first 32 groups; the epilogue is split into two il-halves so the first
half overlaps the tail of the main loop.
"""

import os
import numpy as np

import concourse.bass as bass
import concourse.mybir as mybir
import concourse.tile as tile
from concourse import bacc
from concourse.bass_utils import run_bass_kernel_spmd

F32 = mybir.dt.float32
BF16 = mybir.dt.bfloat16
AOP = mybir.AluOpType
ACTF = mybir.ActivationFunctionType

B, N, D, HN, HC = 8, 256, 16, 64, 64
EPS = 1e-5
NCORES = 8
IPC = N // NCORES          # 32 i-rows per core
NPAIR = B * IPC            # 256 (b,i) pairs per core
NGROUP = NPAIR // 2        # 128 two-pair groups

# F1 packed f32 column offsets
F1_CB1 = 0
F1_CB2 = 1
F1_CWO = 2            # 2:18
F1_NB1 = 18
F1_NB2 = 19
F1_NBO = 20
F1_ABC = 21           # 21:149   (fast path only)
F1_QD = 149           # 149:277  (fast path only)


def _build_program(loop_k: int = 0):
    nc = bacc.Bacc(
        "TRN2",
        target_bir_lowering=False,
        debug=False,
        enable_asserts=False,
        num_devices=1,
    )

    def din(name, shape, dtype=F32):
        return nc.dram_tensor(name, list(shape), dtype, kind="ExternalInput")

    f1w = 277
    d_WA = din("WA", (D, 192), BF16)           # cW1a | cW1b | nW1 (bf16)
    d_xTb = din("xTb", (D, B * N), BF16)       # x^T all nodes
    d_F1 = din("F1", (128, f1w))               # cb|cWoS|nb[|abc|qdiag]
    d_xTmyb = din("xTmyb", (D, NPAIR), BF16)   # core's node cols
    d_Wblk = din("Wblk", (128, 336), BF16)     # W99blk|W01blk|nW2b|nWob
    d_F3 = din("F3", (D, NPAIR))               # BC (cbo*rowsumA)
    d_out = nc.dram_tensor("out_my", [D, NPAIR], F32, kind="ExternalOutput")

    with tile.TileContext(nc) as tc:
        with (
            tc.tile_pool(name="const", bufs=1) as cp,
            tc.tile_pool(name="abc", bufs=1) as ap_,
        ):
            def load(d, shape, dtype=F32):
                t = cp.tile(list(shape), dtype, tag=d.name)
                nc.sync.dma_start(out=t[:, :], in_=d.ap())
                return t

            # ordered by first use; xTb split so chunk 0 lands early
            sWA = load(d_WA, (D, 192), BF16)
            sxTb = cp.tile([D, B * N], BF16, tag="xTb")
            nc.sync.dma_start(out=sxTb[:, 0:768], in_=d_xTb.ap()[:, 0:768])
            sxTmyb = load(d_xTmyb, (D, NPAIR), BF16)
            sF1 = load(d_F1, (128, f1w))
            sWblk = load(d_Wblk, (128, 336), BF16)
            nc.sync.dma_start(
                out=sxTb[:, 768 : B * N], in_=d_xTb.ap()[:, 768 : B * N]
            )
            sF3 = load(d_F3, (D, NPAIR))

            scW1ab = sWA[:, 0:HC]
            scW1bb = sWA[:, HC : 2 * HC]
            snW1b = sWA[:, 128:192]
            scb1d = sF1[:, F1_CB1 : F1_CB1 + 1]
            scb2d = sF1[:, F1_CB2 : F1_CB2 + 1]
            scWoS = sF1[:, F1_CWO : F1_CWO + D]
            snb1 = sF1[0:HN, F1_NB1 : F1_NB1 + 1]
            snb2 = sF1[0:HN, F1_NB2 : F1_NB2 + 1]
            snbo = sF1[0:D, F1_NBO : F1_NBO + 1]
            sW99 = sWblk[:, 0:128]
            sW01 = sWblk[:, 128:256]
            snW2b = sWblk[0:HN, 256:320]
            snWob = sWblk[0:HN, 320:336]
            sBC = sF3[:, 0:NPAIR]

            # ---- prologue ----
            vT = cp.tile([128, B * N], BF16)    # v' dup layout (incl cb1)
            uT = cp.tile([128, NPAIR], F32)     # u dup layout
            uTb = cp.tile([128, NPAIR], BF16)
            U2cb = cp.tile([128, NPAIR], F32)   # 0.01 u@cW2 + cb2 (dup)

            with tc.tile_pool(name="ppro", bufs=2, space="PSUM") as pp:
                # PE p-state warm-up on the first-landed tile (throwaway)
                pwu = pp.tile([64, 192], F32, tag="pw")
                for _ in range(3):
                    nc.tensor.matmul(
                        pwu[:, :], sWA[:, 0:HC], sWA[:, :],
                        start=True, stop=True, tile_position=(0, 0),
                    )

                # v' = cW1b^T @ x + cb1, dup: bottom shifted by N cols
                for ci, c in enumerate(range(0, B * N, 512)):
                    ps = pp.tile([128, 512], F32, tag="pv")
                    nc.tensor.matmul(
                        ps[0:64, :], scW1bb, sxTb[:, c : c + 512],
                        start=True, stop=True, tile_position=(0, 0),
                    )
                    wbot = min(512, B * N - 256 - c)
                    if wbot > 0:
                        nc.tensor.matmul(
                            ps[64:128, 0:wbot], scW1bb,
                            sxTb[:, c + 256 : c + 256 + wbot],
                            start=True, stop=True, tile_position=(0, 64),
                        )
                    # alternate ACT / DVE for the bias-add + bf16 cast
                    if ci % 2 == 0:
                        nc.scalar.activation(
                            vT[:, c : c + 512], ps[:, :],
                            ACTF.Identity, bias=scb1d,
                        )
                    else:
                        nc.vector.tensor_scalar(
                            out=vT[:, c : c + 512], in0=ps[:, :],
                            scalar1=scb1d, scalar2=None, op0=AOP.add,
                        )

                # u = cW1a^T @ xmy, dup: bottom shifted by IPC cols
                psu = pp.tile([128, NPAIR], F32, tag="pp")
                nc.tensor.matmul(
                    psu[0:64, :], scW1ab, sxTmyb[:, :],
                    start=True, stop=True, tile_position=(0, 0),
                )
                nc.tensor.matmul(
                    psu[64:128, 0 : NPAIR - IPC], scW1ab,
                    sxTmyb[:, IPC:NPAIR],
                    start=True, stop=True, tile_position=(0, 64),
                )
                nc.scalar.activation(uT[:, :], psu[:, :], ACTF.Copy)
                nc.vector.tensor_copy(uTb[:, :], uT[:, :])

                # U2cb = blkW01^T @ u + cb2
                psu2 = pp.tile([128, NPAIR], F32, tag="pp")
                nc.tensor.matmul(
                    psu2[:, :], sW01, uTb[:, :], start=True, stop=True,
                )
                nc.scalar.activation(
                    U2cb[:, :], psu2[:, :], ACTF.Identity, bias=scb2d
                )

                # ---- node MLP (bf16, Lrelu-fused)
                pn1 = pp.tile([64, NPAIR], F32, tag="pp")
                nc.tensor.matmul(
                    pn1[:, :], snW1b, sxTmyb[:, :],
                    start=True, stop=True, tile_position=(0, 0),
                )
                h1 = cp.tile([64, NPAIR], BF16)
                nc.scalar.activation(
                    h1[:, :], pn1[:, :], ACTF.Prelu, bias=snb1, alpha=0.01
                )
                pn2 = pp.tile([64, NPAIR], F32, tag="pp")
                nc.tensor.matmul(
                    pn2[:, :], snW2b, h1[:, :],
                    start=True, stop=True, tile_position=(0, 0),
                )
                h2 = cp.tile([64, NPAIR], BF16)
                nc.scalar.activation(
                    h2[:, :], pn2[:, :], ACTF.Prelu, bias=snb2, alpha=0.01
                )
                pn3 = pp.tile([16, NPAIR], F32, tag="pp")
                nc.tensor.matmul(
                    pn3[:, :], snWob, h2[:, :],
                    start=True, stop=True, tile_position=(0, 0),
                )
                accB = cp.tile([D, NPAIR], F32)
                nc.vector.scalar_tensor_tensor(
                    out=accB[:, :], in0=pn3[:, :], scalar=snbo,
                    op0=AOP.add, in1=sBC, op1=AOP.add,
                )

            # ---- main loop + split epilogue ----
            S_mat = cp.tile([128, NGROUP], F32)
            s2_mat = cp.tile([128, NGROUP], F32, tag="s2_mat")
            DVE_MOD = (0, 6, 11)  # il%16 values routed to the DVE path
            import contextlib

            def epilogue_half(h, pe):
                g0 = 64 * h
                if True:
                    sqd_h = sF1[:, F1_QD + g0 : F1_QD + g0 + 64]
                    sab_h = sF1[:, F1_ABC + g0 : F1_ABC + g0 + 64]
                    Sc = cp.tile([128, 64], F32, tag=f"sc{h}")
                    nc.vector.scalar_tensor_tensor(
                        out=Sc[:, :], in0=sqd_h, scalar=-1.0,
                        op0=AOP.mult, in1=S_mat[:, g0 : g0 + 64], op1=AOP.add,
                    )
                    for o in DVE_MOD:
                        nc.vector.scalar_tensor_tensor(
                            out=Sc[:, 4 * o : 4 * o + 4],
                            in0=s2_mat[:, g0 + 4 * o : g0 + 4 * o + 4],
                            scalar=-0.99,
                            op0=AOP.mult,
                            in1=Sc[:, 4 * o : 4 * o + 4],
                            op1=AOP.add,
                        )
                    Sa = cp.tile([128, 64], F32, tag=f"sa{h}")
                    nc.vector.tensor_mul(Sa[:, :], Sc[:, :], sab_h)
                    Sa_ap = Sa[:, :]
                psc1 = pe.tile([16, 64], F32, tag=f"pc1{h}")
                nc.tensor.matmul(
                    psc1[:, :], scWoS[0:64, :], Sa_ap[0:64, :],
                    start=True, stop=True, tile_position=(0, 0),
                )
                psc2 = pe.tile([16, 64], F32, tag=f"pc2{h}")
                nc.tensor.matmul(
                    psc2[:, :], scWoS[64:128, :], Sa_ap[64:128, :],
                    start=True, stop=True, tile_position=(64, 0),
                )
                final = cp.tile([D, 128], F32, tag=f"fin{h}")
                # final col = b*16+il', b = 2q+h2; psc col g' = il'*4+q
                fv = final[:, :].rearrange("p (q h2 i) -> p q h2 i", q=4, h2=2, i=16)
                avh = accB[:, :].rearrange(
                    "p (q h2 i) -> p q h2 i", q=4, h2=2, i=IPC
                )[:, :, :, 16 * h : 16 * h + 16]
                c1v = psc1[:, :].rearrange("p (i q) -> p q i", i=16, q=4)
                c2v = psc2[:, :].rearrange("p (i q) -> p q i", i=16, q=4)
                nc.vector.tensor_add(out=fv[:, :, 0, :], in0=c1v, in1=avh[:, :, 0, :])
                nc.vector.tensor_add(out=fv[:, :, 1, :], in0=c2v, in1=avh[:, :, 1, :])
                dst = d_out.ap().rearrange("p (b i) -> p b i", b=B, i=IPC)[
                    :, :, 16 * h : 16 * h + 16
                ]
                src = final[:, :].rearrange("p (b i) -> p b i", b=B, i=16)
                nc.sync.dma_start(out=dst, in_=src)

            with (
                tc.tile_pool(name="c1p", bufs=20) as c1p,
                tc.tile_pool(name="qp", bufs=20) as qp,
                tc.tile_pool(name="scr", bufs=20) as sp,
                tc.tile_pool(name="pz", bufs=4, space="PSUM") as pzp,
                tc.tile_pool(name="pep", bufs=1, space="PSUM") as pe,
                tc.For_i(0, loop_k, 1) if loop_k > 0 else contextlib.nullcontext(),
            ):
                # bp-major: the first 32 groups only need vT cols 0:512
                for bp in range(4):
                    b0 = 2 * bp
                    for ilh in range(16):  # il pairs (2*ilh, 2*ilh+1)
                        gs = []
                        for il in (2 * ilh, 2 * ilh + 1):
                            g = il * 4 + bp
                            ucol = b0 * IPC + il
                            c1 = c1p.tile([128, N], BF16, tag="c1")
                            nc.vector.tensor_scalar(
                                out=c1[:, :],
                                in0=vT[:, b0 * N : (b0 + 1) * N],
                                scalar1=uT[:, ucol : ucol + 1],
                                scalar2=0.0,
                                op0=AOP.add,
                                op1=AOP.max,
                            )
                            gs.append((g, ucol, c1))
                        # phase-split matmuls: both mm1 (W99), then both mm2
                        pszs = []
                        for (g, ucol, c1) in gs:
                            psz = pzp.tile([128, N], F32, tag="psz")
                            nc.tensor.matmul(
                                psz[:, :], sW99, c1[:, :],
                                start=True, stop=False,
                            )
                            pszs.append(psz)
                        for (g, ucol, c1), psz in zip(gs, pszs):
                            nc.tensor.matmul(
                                psz[:, :], sW01,
                                vT[:, b0 * N : (b0 + 1) * N],
                                start=False, stop=True,
                            )
                        for (g, ucol, c1), psz in zip(gs, pszs):
                            if ((g // 4) % 16) in DVE_MOD:
                                # DVE min-trick path: s1 = sum z2 into S,
                                # s2 = sum min(z2,0); S = s1 - 0.99*s2
                                # applied in the epilogue.
                                wt = qp.tile([128, N], BF16, tag="q")
                                nc.vector.tensor_scalar(
                                    out=wt[:, :], in0=psz[:, :],
                                    scalar1=U2cb[:, ucol : ucol + 1],
                                    scalar2=None, op0=AOP.add, op1=AOP.add,
                                    accum_out=S_mat[:, g : g + 1],
                                )
                                scrap = sp.tile([128, N], BF16, tag="scr")
                                nc.vector.tensor_scalar(
                                    out=scrap[:, :], in0=wt[:, :],
                                    scalar1=0.0, scalar2=None, op0=AOP.min,
                                    op1=AOP.add,
                                    accum_out=s2_mat[:, g : g + 1],
                                )
                                continue
                            q = qp.tile([128, N], BF16, tag="q")
                            nc.scalar.activation(
                                q[:, :], psz[:, :], ACTF.Prelu,
                                bias=U2cb[:, ucol : ucol + 1], alpha=0.01,
                            )
                            scrap = sp.tile([128, N], BF16, tag="scr")
                            nc.vector.tensor_scalar(
                                out=scrap[:, :], in0=q[:, :],
                                scalar1=0.0, scalar2=None, op0=AOP.add,
                                op1=AOP.add,
                                accum_out=S_mat[:, g : g + 1],
                            )
                        if bp == 3 and ilh == 7:
                            epilogue_half(0, pe)
                epilogue_half(1, pe)

    nc.compile()
    return nc


_NC_CACHE = {}


def _get_program():
    loop_k = int(os.environ.get("KERNEL_LOOP", "0"))
    key = ("nc", loop_k)
    if key not in _NC_CACHE:
        _NC_CACHE[key] = _build_program(loop_k)
    return _NC_CACHE[key]


def _leaky_np(v):
    return np.where(v > 0, v, 0.01 * v)


def _prep(x, A_p, nW1, nb1, nW2, nb2, nWo, nbo, cW1, cb1, cW2, cb2, cWo, cbo):
    import ml_dtypes

    f = lambda a: np.ascontiguousarray(np.asarray(a, dtype=np.float32))
    x = f(x)
    A_p = f(A_p)

    # adjacency (stable sigmoid) with suppressed diagonal
    zmat = A_p - np.eye(N, dtype=np.float32) / EPS
    A = np.where(
        zmat >= 0,
        1.0 / (1.0 + np.exp(-np.clip(zmat, -80, 80))),
        np.exp(np.clip(zmat, -80, 80)) / (1.0 + np.exp(np.clip(zmat, -80, 80))),
    ).astype(np.float32)
    A *= 1.0 - np.eye(N, dtype=np.float32)
    rowsum = A.sum(axis=1)

    # fast path: every row constant off-diagonal?
    off = A + np.diag(np.full(N, np.nan, dtype=np.float32))
    rmin = np.nanmin(off, axis=1)
    rmax = np.nanmax(off, axis=1)
    fast_a = bool(np.all(rmax - rmin <= 1e-6 * np.maximum(rmax, 1e-30)))
    a_row = 0.5 * (rmin + rmax)  # per-row constant (if fast_a)

    xT = np.ascontiguousarray(x.reshape(B * N, D).T)  # (16, 2048)
    cW1 = f(cW1)
    cW1a, cW1b = cW1[:D], cW1[D:]
    cW2 = f(cW2)
    cb1 = f(cb1)
    cb2 = f(cb2)
    cWo = f(cWo)
    cbo_f = f(cbo).reshape(D, 1)
    bf = lambda m: np.ascontiguousarray(m).astype(ml_dtypes.bfloat16)

    blk = np.zeros((128, 336), dtype=np.float32)
    blk[:HC, 0:HC] = 0.99 * cW2
    blk[HC:, HC:128] = 0.99 * cW2
    blk[:HC, 128 : 128 + HC] = 0.01 * cW2
    blk[HC:, 128 + HC : 256] = 0.01 * cW2
    blk[0:HN, 256:320] = f(nW2)
    blk[0:HN, 320:336] = f(nWo)

    WA = np.zeros((D, 192), dtype=np.float32)
    WA[:, 0:HC] = cW1a
    WA[:, HC : 2 * HC] = cW1b
    WA[:, 128:192] = f(nW1)

    f1w = 277
    F1s = np.zeros((128, f1w), dtype=np.float32)
    F1s[:, F1_CB1] = np.tile(cb1, 2)
    F1s[:, F1_CB2] = np.tile(cb2, 2)
    F1s[:, F1_CWO : F1_CWO + D] = np.concatenate([cWo, cWo], axis=0)
    F1s[0:HN, F1_NB1] = f(nb1)
    F1s[0:HN, F1_NB2] = f(nb2)
    F1s[0:D, F1_NBO] = f(nbo)

    shared = {
        "WA": bf(WA),
        "xTb": bf(xT),
        "Wblk": bf(blk),
    }

    if not fast_a:
        return None, False
    if True:
        # host-side exact diagonal q_ii per (b,i) pair, in group layout
        xf = x.reshape(B * N, D)
        u_all = xf @ cW1a            # (B*N, HC)
        v_all = xf @ cW1b + cb1      # (B*N, HC)
        c1_diag = _leaky_np(u_all + v_all)
        q_diag = _leaky_np(c1_diag @ cW2 + cb2).reshape(B, N, HC)

    in_maps = []
    for k in range(NCORES):
        i0 = k * IPC
        cols = (np.arange(B)[:, None] * N + (i0 + np.arange(IPC))[None, :]).reshape(-1)
        xTmy = np.ascontiguousarray(xT[:, cols])
        BCm = (cbo_f * np.tile(rowsum[i0 : i0 + IPC], B)[None, :]).astype(np.float32)
        m = dict(shared)
        m["xTmyb"] = bf(xTmy)
        m["F3"] = np.ascontiguousarray(BCm)
        F1 = F1s.copy()
        if fast_a:
            il_ = np.arange(IPC)
            for bp in range(4):
                g = il_ * 4 + bp
                F1[:64, F1_ABC + g] = a_row[i0 + il_][None, :]
                F1[64:, F1_ABC + g] = a_row[i0 + il_][None, :]
                F1[:64, F1_QD + g] = q_diag[2 * bp, i0 + il_, :].T
                F1[64:, F1_QD + g] = q_diag[2 * bp + 1, i0 + il_, :].T
        m["F1"] = np.ascontiguousarray(F1)
        in_maps.append(m)
    return in_maps, fast_a


def kernel(**inputs) -> np.ndarray:
    in_maps, fast_a = _prep(**inputs)
    if not fast_a:
        # rows of A not constant off-diagonal: use the general fallback
        return _kernel_general(**inputs)
    nc = _get_program()
    res = run_bass_kernel_spmd(nc, in_maps, core_ids=list(range(NCORES)))
    out = np.empty((B, N, D), dtype=np.float32)
    for k in range(NCORES):
        i0 = k * IPC
        om = res.results[k]["out_my"]  # (16, 256)
        out[:, i0 : i0 + IPC, :] = om.T.reshape(B, IPC, D)
    return out


# ================= general-A fallback (baseline kernel) =================

def _build_program_general(loop_k: int = 0):
    nc = bacc.Bacc(
        "TRN2",
        target_bir_lowering=False,
        debug=False,
        enable_asserts=False,
        num_devices=1,
    )

    def din(name, shape):
        return nc.dram_tensor(name, list(shape), F32, kind="ExternalInput")

    def dinb(name, shape):
        return nc.dram_tensor(name, list(shape), BF16, kind="ExternalInput")

    d_xT = din("xT", (D, B * N))          # x transposed, all nodes (shared)
    d_xTmy = din("xTmy", (D, NPAIR))      # core's node columns, b-major (per core)
    d_Abc = din("Abc", (128, IPC * N))    # A rows broadcast to 128 partitions (per core)
    d_BC = din("BC", (D, NPAIR))          # cbo x rowsumA chunk (per core)
    d_W99 = dinb("W99", (128, HC))        # 0.99*cW2 stacked twice, bf16
    d_Id64 = dinb("Id64", (128, HC))      # identity stacked twice, bf16
    d_W01 = din("W01", (128, HC))         # 0.01*cW2 stacked twice
    d_cW1a = din("cW1a", (D, HC))
    d_cW1b = din("cW1b", (D, HC))
    d_cb1d = din("cb1d", (128, 1))
    d_cb2d = din("cb2d", (128, 1))
    d_cWo1 = din("cWo1", (128, D))
    d_cWoN99 = din("cWoN99", (128, D))
    d_nW1 = din("nW1", (D, HN))
    d_nb1 = din("nb1", (HN, 1))
    d_nW2 = din("nW2", (HN, HN))
    d_nb2 = din("nb2", (HN, 1))
    d_nWo = din("nWo", (HN, D))
    d_nbo = din("nbo", (D, 1))
    d_out = nc.dram_tensor("out_my", [D, NPAIR], F32, kind="ExternalOutput")

    with tile.TileContext(nc) as tc:
        with (
            tc.tile_pool(name="const", bufs=1) as cp,
            tc.tile_pool(name="abc", bufs=1) as ap_,
        ):
            # ---- load constants / inputs into SBUF
            def load(d, shape, dtype=F32):
                t = cp.tile(list(shape), dtype, tag=d.name)
                nc.sync.dma_start(out=t[:, :], in_=d.ap())
                return t

            sxT = load(d_xT, (D, B * N))
            sxTmy = load(d_xTmy, (D, NPAIR))
            sBC = load(d_BC, (D, NPAIR))
            sW99 = load(d_W99, (128, HC), BF16)
            sId64 = load(d_Id64, (128, HC), BF16)
            sW01 = load(d_W01, (128, HC))
            scW1a = load(d_cW1a, (D, HC))
            scW1b = load(d_cW1b, (D, HC))
            scb1d = load(d_cb1d, (128, 1))
            scb2d = load(d_cb2d, (128, 1))
            scWo1 = load(d_cWo1, (128, D))
            scWoN99 = load(d_cWoN99, (128, D))
            snW1 = load(d_nW1, (D, HN))
            snb1 = load(d_nb1, (HN, 1))
            snW2 = load(d_nW2, (HN, HN))
            snb2 = load(d_nb2, (HN, 1))
            snWo = load(d_nWo, (HN, D))
            snbo = load(d_nbo, (D, 1))

            # A broadcast rows: sliced DMAs so the first groups start early
            sAbc = ap_.tile([128, IPC * N], F32)
            for c in range(0, IPC * N, 1024):
                nc.sync.dma_start(
                    out=sAbc[:, c : c + 1024], in_=d_Abc.ap()[:, c : c + 1024]
                )

            # ---- prologue: v'_dup (128, 2048), uT (128,256), V2_dup, U2cb
            vT = cp.tile([128, B * N], F32)     # top: v'(b,j); bottom: shifted by 256
            uT = cp.tile([128, NPAIR], F32)     # top: u(b,il); bottom: shifted by 32
            V2 = cp.tile([128, B * N], BF16)    # 0.01 * v' @ cW2 (dup layout)
            U2cb = cp.tile([128, NPAIR], F32)   # 0.01 * u @ cW2 + cb2 (dup layout)

            with tc.tile_pool(name="ppro", bufs=2, space="PSUM") as pp:
                # v' = cW1b^T @ xT + cb1  (K=16)
                for c in range(0, B * N, 512):
                    ps = pp.tile([128, 512], F32, tag="pv")
                    nc.tensor.matmul(
                        ps[0:64, :], scW1b[:, :], sxT[:, c : c + 512],
                        start=True, stop=True, tile_position=(0, 0),
                    )
                    wbot = min(512, B * N - 256 - c)
                    if wbot > 0:
                        nc.tensor.matmul(
                            ps[64:128, 0:wbot], scW1b[:, :],
                            sxT[:, c + 256 : c + 256 + wbot],
                            start=True, stop=True, tile_position=(0, 64),
                        )
                    nc.scalar.activation(
                        vT[0:64, c : c + 512], ps[0:64, :],
                        ACTF.Identity, bias=scb1d[0:64, :],
                    )
                    if wbot > 0:
                        nc.scalar.activation(
                            vT[64:128, c : c + wbot], ps[64:128, 0:wbot],
                            ACTF.Identity, bias=scb1d[64:128, :],
                        )

                # u = cW1a^T @ xTmy  (K=16); bottom shifted by 32 cols (next b)
                psu = pp.tile([128, NPAIR], F32, tag="pp")
                nc.tensor.matmul(
                    psu[0:64, :], scW1a[:, :], sxTmy[:, :],
                    start=True, stop=True, tile_position=(0, 0),
                )
                nc.tensor.matmul(
                    psu[64:128, 0 : NPAIR - 32], scW1a[:, :], sxTmy[:, 32:NPAIR],
                    start=True, stop=True, tile_position=(0, 64),
                )
                nc.scalar.activation(uT[0:64, :], psu[0:64, :], ACTF.Copy)
                nc.scalar.activation(
                    uT[64:128, 0 : NPAIR - 32], psu[64:128, 0 : NPAIR - 32], ACTF.Copy
                )

                # V2 = 0.01 * v' @ cW2   (K=64, dup halves via T0 / T10)
                for c in range(0, B * N, 512):
                    ps = pp.tile([128, 512], F32, tag="pv")
                    nc.tensor.matmul(
                        ps[0:64, :], sW01[0:64, :], vT[0:64, c : c + 512],
                        start=True, stop=True, tile_position=(0, 0),
                    )
                    wbot = min(512, B * N - 256 - c)
                    if wbot > 0:
                        nc.tensor.matmul(
                            ps[64:128, 0:wbot], sW01[64:128, :],
                            vT[64:128, c : c + wbot],
                            start=True, stop=True, tile_position=(64, 64),
                        )
                    nc.scalar.activation(V2[0:64, c : c + 512], ps[0:64, :], ACTF.Copy)
                    if wbot > 0:
                        nc.scalar.activation(
                            V2[64:128, c : c + wbot], ps[64:128, 0:wbot], ACTF.Copy
                        )

                # U2cb = 0.01 * u @ cW2 + cb2
                psu2 = pp.tile([128, NPAIR], F32, tag="pp")
                nc.tensor.matmul(
                    psu2[0:64, :], sW01[0:64, :], uT[0:64, :],
                    start=True, stop=True, tile_position=(0, 0),
                )
                nc.tensor.matmul(
                    psu2[64:128, 0 : NPAIR - 32], sW01[64:128, :],
                    uT[64:128, 0 : NPAIR - 32],
                    start=True, stop=True, tile_position=(64, 64),
                )
                nc.scalar.activation(
                    U2cb[0:64, :], psu2[0:64, :], ACTF.Identity, bias=scb2d[0:64, :]
                )
                nc.scalar.activation(
                    U2cb[64:128, 0 : NPAIR - 32], psu2[64:128, 0 : NPAIR - 32],
                    ACTF.Identity, bias=scb2d[64:128, :],
                )

                # ---- node MLP on the core's 256 nodes (all tile T0)
                pn1 = pp.tile([64, NPAIR], F32, tag="pp")
                nc.tensor.matmul(
                    pn1[:, :], snW1[:, :], sxTmy[:, :],
                    start=True, stop=True, tile_position=(0, 0),
                )
                p1s = cp.tile([64, NPAIR], F32)
                nc.scalar.activation(p1s[:, :], pn1[:, :], ACTF.Identity, bias=snb1[:, :])
                h1 = cp.tile([64, NPAIR], F32)
                nc.vector.scalar_tensor_tensor(
                    out=h1[:, :], in0=p1s[:, :], scalar=0.01, in1=p1s[:, :],
                    op0=AOP.mult, op1=AOP.max,
                )
                pn2 = pp.tile([64, NPAIR], F32, tag="pp")
                nc.tensor.matmul(
                    pn2[:, :], snW2[:, :], h1[:, :],
                    start=True, stop=True, tile_position=(0, 0),
                )
                p2s = cp.tile([64, NPAIR], F32)
                nc.scalar.activation(p2s[:, :], pn2[:, :], ACTF.Identity, bias=snb2[:, :])
                h2 = cp.tile([64, NPAIR], F32)
                nc.vector.scalar_tensor_tensor(
                    out=h2[:, :], in0=p2s[:, :], scalar=0.01, in1=p2s[:, :],
                    op0=AOP.mult, op1=AOP.max,
                )
                pn3 = pp.tile([16, NPAIR], F32, tag="pp")
                nc.tensor.matmul(
                    pn3[:, :], snWo[:, :], h2[:, :],
                    start=True, stop=True, tile_position=(0, 0),
                )
                acc = cp.tile([D, NPAIR], F32)
                nc.scalar.activation(acc[:, :], pn3[:, :], ACTF.Identity, bias=snbo[:, :])
                accB = cp.tile([D, NPAIR], F32)
                nc.vector.tensor_add(out=accB[:, :], in0=acc[:, :], in1=sBC[:, :])

            # ---- main loop: 128 groups of 2 pairs
            s1_mat = cp.tile([128, NGROUP], F32)
            s2_mat = cp.tile([128, NGROUP], F32)
            import contextlib

            with (
                tc.tile_pool(name="relu", bufs=4) as rp,
                tc.tile_pool(name="wts", bufs=4) as wp,
                tc.tile_pool(name="scrap", bufs=4) as sp,
                tc.tile_pool(name="pz", bufs=4, space="PSUM") as pzp,
                tc.For_i(0, loop_k, 1) if loop_k > 0 else contextlib.nullcontext(),
            ):
                for il in range(IPC):
                    for bp in range(4):
                        g = il * 4 + bp
                        b0 = 2 * bp
                        ucol = b0 * 32 + il
                        relu_t = rp.tile([128, N], BF16, tag="relu")
                        nc.scalar.activation(
                            relu_t[:, :],
                            vT[:, b0 * N : (b0 + 1) * N],
                            ACTF.Relu,
                            bias=uT[:, ucol : ucol + 1],
                        )
                        psz = pzp.tile([128, N], F32, tag="psz")
                        nc.tensor.matmul(
                            psz[0:64, :], sW99[0:64, :], relu_t[0:64, :],
                            start=True, stop=False, tile_position=(0, 0),
                        )
                        nc.tensor.matmul(
                            psz[0:64, :], sId64[0:64, :],
                            V2[0:64, b0 * N : (b0 + 1) * N],
                            start=False, stop=True, tile_position=(0, 0),
                        )
                        nc.tensor.matmul(
                            psz[64:128, :], sW99[64:128, :], relu_t[64:128, :],
                            start=True, stop=False, tile_position=(64, 64),
                        )
                        nc.tensor.matmul(
                            psz[64:128, :], sId64[64:128, :],
                            V2[64:128, b0 * N : (b0 + 1) * N],
                            start=False, stop=True, tile_position=(64, 64),
                        )
                        wt = wp.tile([128, N], BF16, tag="wt")
                        nc.vector.scalar_tensor_tensor(
                            out=wt[:, :],
                            in0=psz[:, :],
                            scalar=U2cb[:, ucol : ucol + 1],
                            in1=sAbc[:, il * N : (il + 1) * N],
                            op0=AOP.add,
                            op1=AOP.mult,
                            accum_out=s1_mat[:, g : g + 1],
                        )
                        scrap = sp.tile([128, N], BF16, tag="scrap")
                        nc.vector.tensor_scalar(
                            out=scrap[:, :],
                            in0=wt[:, :],
                            scalar1=0.0,
                            scalar2=None,
                            op0=AOP.min,
                            op1=AOP.add,
                            accum_out=s2_mat[:, g : g + 1],
                        )

            # ---- epilogue: coupling = s @ cWo (+ node_out + BC already in accB)
            with tc.tile_pool(name="pep", bufs=2, space="PSUM") as pe:
                psc1 = pe.tile([16, NGROUP], F32, tag="pc")
                nc.tensor.matmul(
                    psc1[:, :], scWo1[0:64, :], s1_mat[0:64, :],
                    start=True, stop=False, tile_position=(0, 0),
                )
                nc.tensor.matmul(
                    psc1[:, :], scWoN99[0:64, :], s2_mat[0:64, :],
                    start=False, stop=True, tile_position=(0, 0),
                )
                psc2 = pe.tile([16, NGROUP], F32, tag="pc")
                nc.tensor.matmul(
                    psc2[:, :], scWo1[64:128, :], s1_mat[64:128, :],
                    start=True, stop=False, tile_position=(64, 0),
                )
                nc.tensor.matmul(
                    psc2[:, :], scWoN99[64:128, :], s2_mat[64:128, :],
                    start=False, stop=True, tile_position=(64, 0),
                )
                final = cp.tile([D, NPAIR], F32)
                # dest col n = b*32+il; group col g = il*4+bp; top: b=2bp, bot: b=2bp+1
                fv = final[:, :].rearrange("p (q h i) -> p q h i", q=4, h=2, i=32)
                av = accB[:, :].rearrange("p (q h i) -> p q h i", q=4, h=2, i=32)
                c1v = psc1[:, :].rearrange("p (i q) -> p q i", i=32, q=4)
                c2v = psc2[:, :].rearrange("p (i q) -> p q i", i=32, q=4)
                nc.vector.tensor_add(out=fv[:, :, 0, :], in0=c1v, in1=av[:, :, 0, :])
                nc.vector.tensor_add(out=fv[:, :, 1, :], in0=c2v, in1=av[:, :, 1, :])
                nc.sync.dma_start(out=d_out.ap(), in_=final[:, :])

    nc.compile()
    return nc


_NC_CACHE_G = {}


def _get_program_general():
    loop_k = int(os.environ.get("KERNEL_LOOP", "0"))
    key = ("nc", loop_k)
    if key not in _NC_CACHE:
        _NC_CACHE_G[key] = _build_program_general(loop_k)
    return _NC_CACHE_G[key]


def _prep_in_maps_general(x, A_p, nW1, nb1, nW2, nb2, nWo, nbo, cW1, cb1, cW2, cb2, cWo, cbo):
    f = lambda a: np.ascontiguousarray(np.asarray(a, dtype=np.float32))
    x = f(x)
    A_p = f(A_p)

    # adjacency (stable sigmoid) with suppressed diagonal
    zmat = A_p - np.eye(N, dtype=np.float32) / EPS
    A = np.where(
        zmat >= 0,
        1.0 / (1.0 + np.exp(-np.clip(zmat, -80, 80))),
        np.exp(np.clip(zmat, -80, 80)) / (1.0 + np.exp(np.clip(zmat, -80, 80))),
    ).astype(np.float32)
    A *= 1.0 - np.eye(N, dtype=np.float32)
    rowsum = A.sum(axis=1)

    xT = np.ascontiguousarray(x.reshape(B * N, D).T)  # (16, 2048)
    cW1 = f(cW1)
    cW1a, cW1b = cW1[:D], cW1[D:]
    cW2 = f(cW2)
    stack2 = lambda m: np.ascontiguousarray(np.concatenate([m, m], axis=0))

    shared = {
        "xT": xT,
        "W01": stack2(0.01 * cW2),
        "cW1a": f(cW1a),
        "cW1b": f(cW1b),
        "cb1d": np.tile(f(cb1).reshape(HC, 1), (2, 1)),
        "cb2d": np.tile(f(cb2).reshape(HC, 1), (2, 1)),
        "cWo1": stack2(f(cWo)),
        "cWoN99": stack2(-0.99 * f(cWo)),
        "nW1": f(nW1),
        "nb1": f(nb1).reshape(HN, 1),
        "nW2": f(nW2),
        "nb2": f(nb2).reshape(HN, 1),
        "nWo": f(nWo),
        "nbo": f(nbo).reshape(D, 1),
    }
    import ml_dtypes

    shared["W99"] = stack2(0.99 * cW2).astype(ml_dtypes.bfloat16)
    shared["Id64"] = stack2(np.eye(HC, dtype=np.float32)).astype(ml_dtypes.bfloat16)

    in_maps = []
    cbo_f = f(cbo).reshape(D, 1)
    for k in range(NCORES):
        i0 = k * IPC
        cols = (np.arange(B)[:, None] * N + (i0 + np.arange(IPC))[None, :]).reshape(-1)
        xTmy = np.ascontiguousarray(xT[:, cols])
        Achunk = A[i0 : i0 + IPC, :]  # (32, 256)
        Abc = np.ascontiguousarray(
            np.broadcast_to(Achunk.reshape(1, IPC * N), (128, IPC * N))
        )
        BC = np.ascontiguousarray(
            cbo_f * np.tile(rowsum[i0 : i0 + IPC], B)[None, :]
        ).astype(np.float32)
        m = dict(shared)
        m["xTmy"] = xTmy
        m["Abc"] = Abc
        m["BC"] = BC
        in_maps.append(m)
    return in_maps


def _kernel_general(**inputs) -> np.ndarray:
    nc = _get_program_general()
    in_maps = _prep_in_maps_general(**inputs)
    res = run_bass_kernel_spmd(nc, in_maps, core_ids=list(range(NCORES)))
    out = np.empty((B, N, D), dtype=np.float32)
    for k in range(NCORES):
        i0 = k * IPC
        om = res.results[k]["out_my"]  # (16, 256)
        out[:, i0 : i0 + IPC, :] = om.T.reshape(B, IPC, D)
    return out

